# revision 1
# baseline (speedup 1.0000x reference)
"""Trainium2 Bass kernel for nn_MicroAdder (tiny dense transformer).

Decomposition: every per-element quantity in the reference network is either
 (a) affine in the basis [u_s, w_s, 1] where u = cos(tok_angle), w = sin(tok_angle)
     (computed with one ACT Sin op each), with position-dependent constant
     coefficients -> ONE PE matmul per 128-row block produces all 8 needed
     per-element linear forms (a, q0, q1, e0, e1, y0, y1, r), including the
     causal-softmax attention mixing (folded into the host-precomputed R matrix), or
 (b) a short elementwise chain (2 rsqrt, 2 relu, ~15 two-input ops) on those forms, or
 (c) the final (T,V) logits expansion  out = L0 (x) E0 + L1 (x) E1, done by a second
     PE matmul per block with a block-diagonal constant rhs.

Sharding: pure data parallel over the batch dim across 8 NeuronCores.
"""

import math
import sys

import numpy as np

for _p in ("/opt/trn_rl_repo", "/root/.axon_site/_ro/trn_rl_repo"):
    if _p not in sys.path:
        sys.path.append(_p)

import concourse.bacc as bacc  # noqa: E402
import concourse.bass as bass  # noqa: E402
import concourse.tile as tile  # noqa: E402
from concourse import mybir  # noqa: E402
from concourse.bass_utils import run_bass_kernel_spmd  # noqa: E402
from concourse.masks import make_identity  # noqa: E402

# ---------------------------------------------------------------- problem dims
B, T, V = 65536, 34, 14
D, EPS, MAX_DIGITS = 5, 1e-5, 10
NCORES = 8
BC = B // NCORES            # rows per core = 8192
P = 128                     # partitions
NPER = BC // P              # rows per partition = 64
NBLK = NPER                 # blocks per core = 64 (block j = rows {p*NPER + j})
SGB = 16                    # blocks per supergroup
NSG = NBLK // SGB           # 4 supergroups
K1 = 2 * T + 1              # basis size = 69
NG = 8                      # matmul1 groups
N1 = NG * T                 # 272
N2 = T * V                  # 476
NPRM = 12

F32 = mybir.dt.float32
I32 = mybir.dt.int32
AF = mybir.ActivationFunctionType
ALU = mybir.AluOpType

# group order in matmul1 output columns (g*T..g*T+T)
G_A, G_Q0, G_Q1, G_E0, G_E1, G_Y0, G_Y1, G_R = range(8)

# PRM slots
(P_SCL, P_BSH, P_RAT, P_SQ0, P_C3, P_H00, P_H10, P_H01, P_H11,
 P_EPS, P_ZERO) = range(11)
CODE_BITS = 24  # idx -> quantized reduced-angle code resolution


# ---------------------------------------------------------------- host tables
def host_tables(tok_A, tok_start, tok_stride, sp_amp, sp_phase, sp_slope, sp_offset,
                norm_w, q_w, q_phase, out_A, out_B, fc1_w, fc2_w, head_w):
    f = np.float64
    A = f(tok_A)
    t = np.arange(T, dtype=f)
    th = 2.0 * np.pi * t / MAX_DIGITS + f(sp_phase)
    pos = np.stack([f(sp_amp) * np.cos(th), f(sp_amp) * np.sin(th),
                    f(sp_slope) * t + f(sp_offset)], axis=-1)
    k = pos @ np.asarray(q_w, f).T
    c0, s0 = np.cos(f(q_phase[0])), np.sin(f(q_phase[0]))
    q = k.copy()
    q[:, 0] = c0 * k[:, 0] - s0 * k[:, 1]
    q[:, 1] = s0 * k[:, 0] + c0 * k[:, 1]
    scores = (q @ k.T) / np.sqrt(f(5.0))
    sm = np.where(np.tril(np.ones((T, T), bool)), scores, -np.inf)
    sm = sm - sm.max(-1, keepdims=True)
    e = np.exp(sm)
    attn = e / e.sum(-1, keepdims=True)

    nw = np.asarray(norm_w, f)
    oA = np.asarray(out_A, f)[:, 0]
    oB = np.asarray(out_B, f)[0]
    S_t = A * A + (pos ** 2).sum(-1)
    rms1 = np.sqrt(S_t / D + EPS)

    M0 = attn * (A * nw[0] * oA[0] / rms1)[None, :]
    M1 = attn * (A * nw[1] * oA[1] / rms1)[None, :]
    c_t = attn @ ((pos * (nw[2:] * oA[2:])[None, :]).sum(-1) / rms1)

    g0 = np.asarray(fc2_w, f)[:, 0]
    g1 = np.asarray(fc2_w, f)[:, 1]
    projs = {
        G_Q0: nw * np.asarray(fc1_w, f)[0],
        G_Q1: nw * np.asarray(fc1_w, f)[1],
        G_E0: 2.0 * g0,
        G_E1: 2.0 * g1,
        G_Y0: nw * np.asarray(head_w, f)[0],
        G_Y1: nw * np.asarray(head_w, f)[1],
    }
    R = np.zeros((K1, NG * T), dtype=f)
    dd = np.eye(T, dtype=f)
    for gi in range(NG):
        cols = slice(gi * T, (gi + 1) * T)
        if gi == G_A:
            R[0:T, cols] = M0.T
            R[T:2 * T, cols] = M1.T
            R[2 * T, cols] = c_t
        elif gi == G_R:
            b2 = (oB ** 2).sum()
            R[0:T, cols] = 2 * A * oB[0] * dd + b2 * M0.T
            R[T:2 * T, cols] = 2 * A * oB[1] * dd + b2 * M1.T
            R[2 * T, cols] = 2 * (pos * oB[None, 2:]).sum(-1) + b2 * c_t
        else:
            v = projs[gi]
            bv = (oB * v).sum()
            R[0:T, cols] = A * v[0] * dd + bv * M0.T
            R[T:2 * T, cols] = A * v[1] * dd + bv * M1.T
            R[2 * T, cols] = (pos * v[None, 2:]).sum(-1) + bv * c_t

    # Basis change for the half-angle scheme actually computed on device:
    #   device computes u' = sin^2(phi), w' = sin(phi)*cos(phi) where
    #   ang = 2*phi + pi  ->  cos(ang) = 2u' - 1, sin(ang) = -2w'.
    # Old basis rows: u = cos(ang), w = sin(ang).
    Rn = R.copy()
    Rn[0:T, :] = 2.0 * R[0:T, :]
    Rn[T:2 * T, :] = -2.0 * R[T:2 * T, :]
    Rn[2 * T, :] = R[2 * T, :] - R[0:T, :].sum(axis=0)
    R = Rn

    G00, G01, G11 = (g0 * g0).sum(), (g0 * g1).sum(), (g1 * g1).sum()
    if G00 > 1e-30:
        sq0, rat = np.sqrt(G00), G01 / G00
        c3 = np.sqrt(max(G11 - G01 * G01 / G00, 0.0))
    else:
        sq0, rat, c3 = 0.0, 0.0, np.sqrt(G11)
    hv0 = nw * np.asarray(head_w, f)[0]
    hv1 = nw * np.asarray(head_w, f)[1]
    H = np.array([[(g0 * hv0).sum(), (g0 * hv1).sum()],
                  [(g1 * hv0).sum(), (g1 * hv1).sum()]])

    dvoc = np.arange(V, dtype=f)
    ang = f(tok_start) + dvoc * f(tok_stride)
    E = np.stack([A * np.cos(ang), A * np.sin(ang)], axis=-1)
    RHS2 = np.zeros((2 * T, N2), dtype=f)
    for t_ in range(T):
        RHS2[t_, t_ * V:(t_ + 1) * V] = E[:, 0]
        RHS2[T + t_, t_ * V:(t_ + 1) * V] = E[:, 1]

    # idx -> code LUT: reduced angle, quantized to CODE_BITS
    angv = np.mod(f(tok_start) + np.arange(V, dtype=f) * f(tok_stride), 2 * np.pi)
    CODE = np.clip(np.round(angv * (2 ** CODE_BITS) / (2 * np.pi)),
                   0, 2 ** CODE_BITS - 1).astype(np.int32)
    half_scale = (2 * np.pi / (2 ** CODE_BITS)) / 2.0  # phi = code*hs - pi/2

    SROW = np.tile(S_t, SGB)[None, :]  # [1, 544]
    PRM = np.zeros((1, NPRM), dtype=f)
    PRM[0, P_SCL] = half_scale
    PRM[0, P_BSH] = -np.pi / 2.0
    PRM[0, P_RAT] = rat
    PRM[0, P_SQ0] = sq0
    PRM[0, P_C3] = c3
    PRM[0, P_H00] = H[0, 0]
    PRM[0, P_H10] = H[1, 0]
    PRM[0, P_H01] = H[0, 1]
    PRM[0, P_H11] = H[1, 1]
    PRM[0, P_EPS] = EPS
    PRM[0, P_ZERO] = 0.0
    return (R.astype(np.float32), RHS2.astype(np.float32),
            np.ascontiguousarray(SROW, np.float32).copy(),
            np.ascontiguousarray(PRM, np.float32).copy(), CODE)


# ---------------------------------------------------------------- bass kernel
def build_bass():
    nc = bacc.Bacc("TRN2", target_bir_lowering=False, debug=False)

    idx_d = nc.dram_tensor("idx", [BC, T], I32, kind="ExternalInput").ap()
    r_d = nc.dram_tensor("R", [K1, N1], F32, kind="ExternalInput").ap()
    rhs2_d = nc.dram_tensor("RHS2", [2 * T, N2], F32, kind="ExternalInput").ap()
    srow_d = nc.dram_tensor("SROW", [1, T * SGB], F32, kind="ExternalInput").ap()
    prm_d = nc.dram_tensor("PRM", [1, NPRM], F32, kind="ExternalInput").ap()
    out_d = nc.dram_tensor("out", [BC, N2], F32, kind="ExternalOutput").ap()

    # DRAM views: partition p holds rows p*NPER .. p*NPER+NPER-1
    idx_v = idx_d.rearrange("(p n) t -> p n t", p=P)       # [128, 64, 34]
    out_v = out_d.rearrange("(p n) c -> p n c", p=P)       # [128, 64, 476]

    FW = T * SGB  # 544 columns per supergroup

    with tile.TileContext(nc) as tc:
        with (
            tc.tile_pool(name="const", bufs=1) as cpool,
            tc.tile_pool(name="idxp", bufs=2) as idxp,
            tc.tile_pool(name="idxf", bufs=2) as idxfp,
            tc.tile_pool(name="uw", bufs=2) as uwp,
            tc.tile_pool(name="dt", bufs=2) as dtp,
            tc.tile_pool(name="lt2", bufs=2) as lt2p,
            tc.tile_pool(name="stage", bufs=2) as stp,
            tc.tile_pool(name="mt", bufs=2) as mtp,
            tc.tile_pool(name="outsb", bufs=3) as outp,
            tc.tile_pool(name="ptp", bufs=2, space="PSUM") as ptp,
            tc.tile_pool(name="pmm1", bufs=2, space="PSUM") as pmm1p,
            tc.tile_pool(name="pout", bufs=2, space="PSUM") as poutp,
        ):
            # ---- constants
            ident = cpool.tile([P, P], F32)
            make_identity(nc, ident[:])
            r_sb = cpool.tile([K1, N1], F32)
            nc.sync.dma_start(r_sb[:], r_d)
            rhs2_sb = cpool.tile([2 * T, N2], F32)
            nc.sync.dma_start(rhs2_sb[:], rhs2_d)
            s_sb = cpool.tile([P, FW], F32)
            nc.sync.dma_start(s_sb[:], srow_d.broadcast_to([P, FW]))
            prm_sb = cpool.tile([P, NPRM], F32)
            nc.sync.dma_start(prm_sb[:], prm_d.broadcast_to([P, NPRM]))

            def prm(i):
                return prm_sb[:, i:i + 1]

            for sg in range(NSG):
                j0 = sg * SGB
                # ---------------- phase A: idx -> u,w basis (interleaved 69-stride)
                idx_t = idxp.tile([P, FW], I32, tag="idx")
                nc.sync.dma_start(idx_t[:], idx_v[:, j0:j0 + SGB, :])
                idxf_t = idxfp.tile([P, FW], F32, tag="idxf")
                nc.vector.tensor_copy(idxf_t[:], idx_t[:])
                uw_t = uwp.tile([P, SGB * K1], F32, tag="uw")   # [128, 16*69]
                uw3 = uw_t[:].rearrange("p (j k) -> p j k", k=K1)
                # half-angle scheme: phi = code*hs - pi/2 in [-pi/2, pi/2]
                # sh = sin(phi); chh = cos(phi) = sin(phi + pi/2) (arg in [0, pi])
                # basis: u' = sh^2, w' = sh*chh
                sh_t = idxfp.tile([P, FW], F32, tag="sh")
                nc.scalar.activation(sh_t[:], idxf_t[:], AF.Sin,
                                     bias=prm(P_BSH), scale=prm(P_SCL))
                chh_t = idxfp.tile([P, FW], F32, tag="chh")
                nc.scalar.activation(chh_t[:], idxf_t[:], AF.Sin,
                                     bias=prm(P_ZERO), scale=prm(P_SCL))
                sh3 = sh_t[:].rearrange("p (j t) -> p j t", t=T)
                chh3 = chh_t[:].rearrange("p (j t) -> p j t", t=T)
                nc.scalar.activation(uw3[:, :, 0:T], sh3, AF.Square,
                                     bias=prm(P_ZERO), scale=1.0)
                nc.vector.tensor_mul(uw3[:, :, T:2 * T], sh3, chh3)
                nc.vector.memset(uw3[:, :, 2 * T:K1], 1.0)

                # ---------------- phase B/C: per-block transpose+matmul1, per-pair drains
                ar_t = stp.tile([P, FW], F32, tag="ar")
                rho_t = stp.tile([P, SGB * 68], F32, tag="rho")
                tab_t = stp.tile([P, SGB * 68], F32, tag="tab")
                yr_t = stp.tile([P, SGB * 102], F32, tag="yr")
                rho3 = rho_t[:].rearrange("p (j c) -> p j c", c=68)
                tab3 = tab_t[:].rearrange("p (j c) -> p j c", c=68)
                yr3 = yr_t[:].rearrange("p (j c) -> p j c", c=102)
                ar3 = ar_t[:].rearrange("p (j t) -> p j t", t=T)

                pt1 = None
                for j in range(SGB):
                    c4 = j % 4
                    if c4 == 0:
                        pt1 = ptp.tile([K1, 4 * P], F32, tag="tp")
                    nc.tensor.transpose(pt1[:, c4 * P:(c4 + 1) * P],
                                        uw3[:, j, :], ident[:])
                    if c4 == 3:
                        dt_t = dtp.tile([K1, 4 * P], F32, tag="dt")
                        nc.vector.tensor_copy(dt_t[:], pt1[:])
                        # matmul1 + drains for the two pairs in these 4 blocks
                        for h in range(2):
                            jj = j - 3 + 2 * h      # first block of pair
                            pr = (jj - j0 * 0) // 2  # pair idx within sg: jj is local
                            pm = pmm1p.tile([P, 1024], F32, tag="mm1")
                            pm3 = pm[:].rearrange("p (b c) -> p b c", b=2)
                            for b in range(2):
                                nc.tensor.matmul(
                                    pm3[:, b, 0:N1],
                                    dt_t[:, (2 * h + b) * P:(2 * h + b + 1) * P],
                                    r_sb[:],
                                    start=True, stop=True)

                            def g(gi):
                                return pm3[:, :, gi * T:(gi + 1) * T]

                            pj = jj  # local block index of first-in-pair
                            # drain y0,y1,r (adjacent groups) to SBUF first
                            nc.vector.tensor_copy(
                                yr3[:, pj:pj + 2, :],
                                pm3[:, :, G_Y0 * T:(G_R + 1) * T])
                            nc.vector.tensor_mul(
                                ar3[:, pj:pj + 2, :], g(G_A),
                                yr3[:, pj:pj + 2, 2 * T:3 * T])
                            nc.vector.tensor_scalar_max(
                                rho3[:, pj:pj + 2, :],
                                pm3[:, :, G_Q0 * T:(G_Q1 + 1) * T], 0.0)
                            nc.vector.tensor_mul(
                                tab3[:, pj:pj + 2, 0:T],
                                rho3[:, pj:pj + 2, 0:T], g(G_E0))
                            nc.vector.tensor_mul(
                                tab3[:, pj:pj + 2, T:2 * T],
                                rho3[:, pj:pj + 2, T:2 * T], g(G_E1))

                # ---------------- phase D: supergroup elementwise chain [128, 544]
                rho0 = rho3[:, :, 0:T]
                rho1 = rho3[:, :, T:2 * T]
                y0v = yr3[:, :, 0:T]
                y1v = yr3[:, :, T:2 * T]

                n2 = mtp.tile([P, FW], F32, tag="n2")
                nc.vector.tensor_add(n2[:], ar_t[:], s_sb[:])
                s2 = mtp.tile([P, FW], F32, tag="s2")
                nc.scalar.activation(s2[:], n2[:], AF.Sqrt, bias=prm(P_EPS),
                                     scale=1.0 / D)
                inv2 = mtp.tile([P, FW], F32, tag="inv2")
                nc.vector.reciprocal(inv2[:], s2[:])

                z0 = mtp.tile([P, FW], F32, tag="z0")
                nc.gpsimd.tensor_mul(z0[:], rho0, inv2[:])
                z1 = mtp.tile([P, FW], F32, tag="z1")
                nc.gpsimd.tensor_mul(z1[:], rho1, inv2[:])
                tau = mtp.tile([P, FW], F32, tag="tau")
                nc.gpsimd.tensor_add(tau[:], tab3[:, :, 0:T], tab3[:, :, T:2 * T])
                it2 = mtp.tile([P, FW], F32, tag="it2")
                nc.gpsimd.tensor_mul(it2[:], tau[:], inv2[:])

                v1 = mtp.tile([P, FW], F32, tag="v1")
                nc.vector.scalar_tensor_tensor(v1[:], z1[:], prm(P_RAT), z0[:],
                                               op0=ALU.mult, op1=ALU.add)
                v1sq = mtp.tile([P, FW], F32, tag="v1sq")
                nc.scalar.activation(v1sq[:], v1[:], AF.Square,
                                     bias=prm(P_ZERO), scale=prm(P_SQ0))
                v2sq = mtp.tile([P, FW], F32, tag="v2sq")
                nc.scalar.activation(v2sq[:], z1[:], AF.Square,
                                     bias=prm(P_ZERO), scale=prm(P_C3))

                n3 = mtp.tile([P, FW], F32, tag="n3")
                nc.vector.tensor_add(n3[:], n2[:], it2[:])
                nc.gpsimd.tensor_add(n3[:], n3[:], v1sq[:])
                nc.vector.tensor_add(n3[:], n3[:], v2sq[:])
                s3 = mtp.tile([P, FW], F32, tag="s3")
                nc.scalar.activation(s3[:], n3[:], AF.Sqrt, bias=prm(P_EPS),
                                     scale=1.0 / D)
                inv3 = mtp.tile([P, FW], F32, tag="inv3")
                nc.vector.reciprocal(inv3[:], s3[:])

                lint = mtp.tile([P, SGB * 68], F32, tag="lint")
                lint3 = lint[:].rearrange("p (j c) -> p j c", c=68)
                p0 = mtp.tile([P, FW], F32, tag="p0")
                nc.vector.scalar_tensor_tensor(p0[:], z1[:], prm(P_H10), y0v,
                                               op0=ALU.mult, op1=ALU.add)
                nc.vector.scalar_tensor_tensor(p0[:], z0[:], prm(P_H00), p0[:],
                                               op0=ALU.mult, op1=ALU.add)
                nc.gpsimd.tensor_mul(lint3[:, :, 0:T], p0[:], inv3[:])
                p1 = mtp.tile([P, FW], F32, tag="p1")
                nc.vector.scalar_tensor_tensor(p1[:], z1[:], prm(P_H11), y1v,
                                               op0=ALU.mult, op1=ALU.add)
                nc.vector.scalar_tensor_tensor(p1[:], z0[:], prm(P_H01), p1[:],
                                               op0=ALU.mult, op1=ALU.add)
                nc.gpsimd.tensor_mul(lint3[:, :, T:2 * T], p1[:], inv3[:])

                # ---------------- phase F: final expansion + store
                pt2 = None
                for j in range(SGB):
                    c4 = j % 4
                    if c4 == 0:
                        pt2 = ptp.tile([2 * T, 4 * P], F32, tag="tp")
                    nc.tensor.transpose(pt2[:, c4 * P:(c4 + 1) * P],
                                        lint3[:, j, :], ident[:])
                    if c4 == 3:
                        lt2_t = lt2p.tile([2 * T, 4 * P], F32, tag="lt2")
                        nc.vector.tensor_copy(lt2_t[:], pt2[:])
                        for b in range(4):
                            jb = j - 3 + b
                            po = poutp.tile([P, N2], F32, tag="po")
                            nc.tensor.matmul(po[:], lt2_t[:, b * P:(b + 1) * P],
                                             rhs2_sb[:], start=True, stop=True)
                            o_sb = outp.tile([P, N2], F32, tag="osb")
                            if jb % 2 == 0:
                                nc.vector.tensor_copy(o_sb[:], po[:])
                            else:
                                nc.scalar.copy(o_sb[:], po[:])
                            nc.sync.dma_start(out_v[:, j0 + jb, :], o_sb[:])

    nc.compile()
    return nc


_CACHE = {}


def _get_nc():
    if "nc" not in _CACHE:
        _CACHE["nc"] = build_bass()
    return _CACHE["nc"]


def kernel(**inputs) -> np.ndarray:
    idx = np.asarray(inputs["idx"]).astype(np.int32)
    kw = {k: np.asarray(v, np.float64) for k, v in inputs.items() if k != "idx"}
    R, RHS2, SROW, PRM, CODE = host_tables(**kw)
    idxc = np.ascontiguousarray(CODE[idx])  # remap token id -> angle code
    nc = _get_nc()
    in_maps = [
        {"idx": idxc[c * BC:(c + 1) * BC], "R": R, "RHS2": RHS2,
         "SROW": SROW, "PRM": PRM}
        for c in range(NCORES)
    ]
    res = run_bass_kernel_spmd(nc, in_maps, core_ids=list(range(NCORES)))
    out = np.concatenate([res.results[c]["out"] for c in range(NCORES)], axis=0)
    return np.ascontiguousarray(out.reshape(B, T, V).astype(np.float32))



# revision 10
# speedup vs baseline: 2.1786x; 2.1786x over previous
"""Trainium2 Bass kernel for nn_MicroAdder (tiny dense transformer).

Decomposition (v2, fp16 PE path):
 - Host precomputes u_v=cos(ang_v), w_v=sin(ang_v) per vocab entry and expands
   idx -> basis [u_s, w_s, 1] (69 wide) directly into the fp16 "uw" input, so
   no trig runs on-device.
 - Per 128-row block: PE transpose basis -> [69,128], then two fp16 matmuls
   against constant matrices emit 8 per-element linear forms:
   QEY = (q0,q1,e0,e1,y0,y1) [204 cols] and AR = (alpha, r) [68 cols].
 - A short elementwise chain (2 abs_rsqrt, 2 square, ~15 mul/add) computes the
   two rmsnorm denominators and the head-plane values s0,s1 per (row,t).
 - PE transpose s -> [68,128], fp16 matmul vs block-diagonal RHS2 expands to
   logits [128,476]; fp16 copy to SBUF; fp16 DMA out; host casts to fp32.

Sharding: pure data parallel over the batch dim across 8 NeuronCores.
"""

import sys

import numpy as np

for _p in ("/opt/trn_rl_repo", "/root/.axon_site/_ro/trn_rl_repo"):
    if _p not in sys.path:
        sys.path.append(_p)

import concourse.bacc as bacc  # noqa: E402
import concourse.tile as tile  # noqa: E402
from concourse import mybir  # noqa: E402
from concourse.bass_utils import run_bass_kernel_spmd  # noqa: E402
from concourse.masks import make_identity  # noqa: E402

# ---------------------------------------------------------------- problem dims
B, T, V = 65536, 34, 14
D, EPS, MAX_DIGITS = 5, 1e-5, 10
NCORES = 8
BC = B // NCORES            # rows per core = 8192
P = 128                     # partitions
NBLK = BC // P              # blocks per core = 64 (block j = rows {p*NBLK + j})
SGB = 32                    # blocks per supergroup
NSG = NBLK // SGB           # 2 supergroups
K1 = 2 * T + 1              # basis size = 69
NQEY = 7 * T                # 238: q0,q1,e0,e1,y0,y1,alpha
NAR = T                     # 34: r
N2 = T * V                  # 476
NPRM = 10

F32 = mybir.dt.float32
F16 = mybir.dt.float16
AF = mybir.ActivationFunctionType
ALU = mybir.AluOpType

# PRM slots
P_RAT, P_SQ0, P_C3, P_H00, P_H10, P_H01, P_H11, P_EPS, P_ZERO, P_UNUSED = range(10)


# ---------------------------------------------------------------- host tables
def host_tables(tok_A, tok_start, tok_stride, sp_amp, sp_phase, sp_slope, sp_offset,
                norm_w, q_w, q_phase, out_A, out_B, fc1_w, fc2_w, head_w):
    f = np.float64
    A = f(tok_A)
    t = np.arange(T, dtype=f)
    th = 2.0 * np.pi * t / MAX_DIGITS + f(sp_phase)
    pos = np.stack([f(sp_amp) * np.cos(th), f(sp_amp) * np.sin(th),
                    f(sp_slope) * t + f(sp_offset)], axis=-1)
    k = pos @ np.asarray(q_w, f).T
    c0, s0 = np.cos(f(q_phase[0])), np.sin(f(q_phase[0]))
    q = k.copy()
    q[:, 0] = c0 * k[:, 0] - s0 * k[:, 1]
    q[:, 1] = s0 * k[:, 0] + c0 * k[:, 1]
    scores = (q @ k.T) / np.sqrt(f(5.0))
    sm = np.where(np.tril(np.ones((T, T), bool)), scores, -np.inf)
    sm = sm - sm.max(-1, keepdims=True)
    e = np.exp(sm)
    attn = e / e.sum(-1, keepdims=True)

    nw = np.asarray(norm_w, f)
    oA = np.asarray(out_A, f)[:, 0]
    oB = np.asarray(out_B, f)[0]
    S_t = A * A + (pos ** 2).sum(-1)
    rms1 = np.sqrt(S_t / D + EPS)

    M0 = attn * (A * nw[0] * oA[0] / rms1)[None, :]
    M1 = attn * (A * nw[1] * oA[1] / rms1)[None, :]
    c_t = attn @ ((pos * (nw[2:] * oA[2:])[None, :]).sum(-1) / rms1)

    g0 = np.asarray(fc2_w, f)[:, 0]
    g1 = np.asarray(fc2_w, f)[:, 1]

    dd = np.eye(T, dtype=f)
    b2 = (oB ** 2).sum()

    def lin_group(v):
        """Columns (69, T) for per-element value <v, x_t> with v a model-dim
        vector; x includes the rank-1 attention update."""
        g = np.zeros((K1, T), dtype=f)
        bv = (oB * v).sum()
        g[0:T] = A * v[0] * dd + bv * M0.T
        g[T:2 * T] = A * v[1] * dd + bv * M1.T
        g[2 * T] = (pos * v[None, 2:]).sum(-1) + bv * c_t
        return g

    def alpha_group():
        g = np.zeros((K1, T), dtype=f)
        g[0:T] = M0.T
        g[T:2 * T] = M1.T
        g[2 * T] = c_t
        return g

    def r_group():
        # r = 2<x_emb, oB> + b2 * alpha  (so that alpha*r = |x|^2 - |x_emb|^2)
        g = np.zeros((K1, T), dtype=f)
        g[0:T] = 2 * A * oB[0] * dd + b2 * M0.T
        g[T:2 * T] = 2 * A * oB[1] * dd + b2 * M1.T
        g[2 * T] = 2 * (pos * oB[None, 2:]).sum(-1) + b2 * c_t
        return g

    RQEY = np.concatenate([
        lin_group(nw * np.asarray(fc1_w, f)[0]),      # q0
        lin_group(nw * np.asarray(fc1_w, f)[1]),      # q1
        lin_group(2.0 * g0),                          # e0
        lin_group(2.0 * g1),                          # e1
        lin_group(nw * np.asarray(head_w, f)[0]),     # y0
        lin_group(nw * np.asarray(head_w, f)[1]),     # y1
        alpha_group(),                                # alpha
    ], axis=1)
    RAR = r_group()

    G00, G01, G11 = (g0 * g0).sum(), (g0 * g1).sum(), (g1 * g1).sum()
    if G00 > 1e-30:
        sq0, rat = np.sqrt(G00), G01 / G00
        c3 = np.sqrt(max(G11 - G01 * G01 / G00, 0.0))
    else:
        sq0, rat, c3 = 0.0, 0.0, np.sqrt(G11)
    hv0 = nw * np.asarray(head_w, f)[0]
    hv1 = nw * np.asarray(head_w, f)[1]
    H = np.array([[(g0 * hv0).sum(), (g0 * hv1).sum()],
                  [(g1 * hv0).sum(), (g1 * hv1).sum()]])

    dvoc = np.arange(V, dtype=f)
    ang = f(tok_start) + dvoc * f(tok_stride)
    E = np.stack([A * np.cos(ang), A * np.sin(ang)], axis=-1)
    RHS2 = np.zeros((2 * T, N2), dtype=f)
    for t_ in range(T):
        RHS2[t_, t_ * V:(t_ + 1) * V] = E[:, 0]
        RHS2[T + t_, t_ * V:(t_ + 1) * V] = E[:, 1]

    SROW = np.tile(S_t, SGB)[None, :]  # [1, SGB*T]
    PRM = np.zeros((1, NPRM), dtype=np.float64)
    PRM[0, P_RAT] = rat
    PRM[0, P_SQ0] = sq0
    PRM[0, P_C3] = c3
    PRM[0, P_H00] = H[0, 0]
    PRM[0, P_H10] = H[1, 0]
    PRM[0, P_H01] = H[0, 1]
    PRM[0, P_H11] = H[1, 1]
    PRM[0, P_EPS] = EPS
    PRM[0, P_ZERO] = 0.0

    LUTU = np.cos(ang).astype(np.float32)
    LUTW = np.sin(ang).astype(np.float32)
    return (RQEY.astype(np.float16), RAR.astype(np.float16),
            RHS2.astype(np.float16), SROW.astype(np.float16).copy(),
            PRM.astype(np.float32).copy(), LUTU, LUTW)


# ---------------------------------------------------------------- bass kernel
def build_bass():
    nc = bacc.Bacc("TRN2", target_bir_lowering=False, debug=False)

    uw_d = nc.dram_tensor("uw", [P, NBLK, K1], F16, kind="ExternalInput").ap()
    rqey_d = nc.dram_tensor("rqey", [K1, NQEY], F16, kind="ExternalInput").ap()
    rar_d = nc.dram_tensor("rar", [K1, NAR], F16, kind="ExternalInput").ap()
    rhs2_d = nc.dram_tensor("rhs2", [2 * T, N2], F16, kind="ExternalInput").ap()
    srow_d = nc.dram_tensor("srow", [1, SGB * T], F16, kind="ExternalInput").ap()
    prm_d = nc.dram_tensor("prm", [1, NPRM], F32, kind="ExternalInput").ap()
    out_d = nc.dram_tensor("out", [BC, N2], F16, kind="ExternalOutput").ap()

    out_v = out_d.rearrange("(p n) c -> p n c", p=P)       # [128, 64, 476]

    E1 = SGB * T  # 1088 chain width per supergroup

    with tile.TileContext(nc) as tc:
        with (
            tc.tile_pool(name="const", bufs=1) as cpool,
            tc.tile_pool(name="uwp", bufs=2) as uwp,
            tc.tile_pool(name="dtp", bufs=2) as dtp,
            tc.tile_pool(name="stg", bufs=2) as stgp,
            tc.tile_pool(name="chn", bufs=2) as chnp,
            tc.tile_pool(name="outsb", bufs=4) as outp,
            tc.tile_pool(name="ptp", bufs=1, space="PSUM") as ptp,
            tc.tile_pool(name="qeyp", bufs=2, space="PSUM") as qeyp,
            tc.tile_pool(name="arp", bufs=1, space="PSUM") as arp,
            tc.tile_pool(name="pout", bufs=2, space="PSUM") as poutp,
        ):
            # ---- constants
            ident = cpool.tile([P, P], F16)
            make_identity(nc, ident[:])
            rqey_sb = cpool.tile([K1, NQEY], F16)
            nc.sync.dma_start(rqey_sb[:], rqey_d)
            rar_sb = cpool.tile([K1, NAR], F16)
            nc.sync.dma_start(rar_sb[:], rar_d)
            rhs2_sb = cpool.tile([2 * T, N2], F16)
            nc.sync.dma_start(rhs2_sb[:], rhs2_d)
            srow_sb = cpool.tile([P, SGB * T], F16)
            nc.sync.dma_start(srow_sb[:], srow_d.broadcast_to([P, SGB * T]))
            prm_sb = cpool.tile([P, NPRM], F32)
            nc.sync.dma_start(prm_sb[:], prm_d.broadcast_to([P, NPRM]))

            def prm(i):
                return prm_sb[:, i:i + 1]

            for sg in range(NSG):
                j0 = sg * SGB
                uw_t = uwp.tile([P, SGB, K1], F16, tag="uw")
                nc.sync.dma_start(uw_t[:], uw_d[:, j0:j0 + SGB, :])

                rho_t = stgp.tile([P, SGB, 2 * T], F16, tag="rho")
                ey_t = stgp.tile([P, SGB, 5 * T], F16, tag="eyt")  # e0,e1,y0,y1,alpha
                ar_t = stgp.tile([P, SGB, T], F16, tag="art")

                # ---------------- phase B: transpose + matmul1 + drains (per 4 blocks)
                for g in range(SGB // 4):
                    pt1 = ptp.tile([K1, 4 * P], F16, tag="tp")
                    for b in range(4):
                        nc.tensor.transpose(pt1[:, b * P:(b + 1) * P],
                                            uw_t[:, g * 4 + b, :], ident[:])
                    dt = dtp.tile([K1, 4 * P], F16, tag="dt")
                    if g % 2 == 0:
                        nc.vector.tensor_copy(dt[:], pt1[:])
                    else:
                        nc.scalar.copy(dt[:], pt1[:])
                    # per-block slot padded to 256 els (1024B) so each matmul
                    # output stays inside one PSUM bank
                    qey = qeyp.tile([P, 4, 256], F32, tag="qey")
                    arq = arp.tile([P, 4 * NAR], F32, tag="ar")
                    qv = qey[:, :, 0:NQEY]                      # [P, 4, 204]
                    av = arq[:].rearrange("p (b c) -> p b c", b=4)  # [P, 4, 68]
                    for b in range(4):
                        nc.tensor.matmul(qey[:, b, 0:NQEY],
                                         dt[:, b * P:(b + 1) * P], rqey_sb[:],
                                         start=True, stop=True)
                        nc.tensor.matmul(arq[:, b * NAR:(b + 1) * NAR],
                                         dt[:, b * P:(b + 1) * P], rar_sb[:],
                                         start=True, stop=True)
                    sl = slice(g * 4, g * 4 + 4)
                    # rho = relu(q01)  [ACT]
                    nc.scalar.activation(rho_t[:, sl, :], qv[:, :, 0:2 * T], AF.Relu,
                                         bias=prm(P_ZERO), scale=1.0)
                    # e0,e1,y0,y1,alpha copy  [ACT]
                    nc.scalar.copy(ey_t[:, sl, :], qv[:, :, 2 * T:7 * T])
                    # ar = alpha(SBUF) * r(PSUM)  [DVE]
                    nc.vector.tensor_mul(ar_t[:, sl, :], ey_t[:, sl, 4 * T:5 * T],
                                         av[:, :, 0:T])

                # ---------------- phase D: elementwise chain [P, SGB, T]
                n2 = chnp.tile([P, SGB, T], F16, tag="n2")
                nc.vector.tensor_add(n2[:], ar_t[:],
                                     srow_sb[:].rearrange("p (a t) -> p a t", t=T))
                inv2 = chnp.tile([P, SGB, T], F16, tag="inv2")
                nc.scalar.activation(inv2[:], n2[:], AF.Abs_reciprocal_sqrt,
                                     bias=prm(P_EPS), scale=1.0 / D)
                z = chnp.tile([P, SGB, 2 * T], F16, tag="z")
                nc.vector.tensor_mul(z[:, :, 0:T], rho_t[:, :, 0:T], inv2[:])
                nc.vector.tensor_mul(z[:, :, T:2 * T], rho_t[:, :, T:2 * T], inv2[:])
                ze = chnp.tile([P, SGB, 2 * T], F16, tag="ze")
                nc.vector.tensor_mul(ze[:], z[:], ey_t[:, :, 0:2 * T])
                it2 = chnp.tile([P, SGB, T], F16, tag="it2")
                nc.gpsimd.tensor_add(it2[:], ze[:, :, 0:T], ze[:, :, T:2 * T])

                v1 = chnp.tile([P, SGB, T], F16, tag="v1")
                nc.vector.affine_then_add(v1[:], z[:, :, T:2 * T], z[:, :, 0:T],
                                          scale=prm(P_RAT), bias=0.0)
                v1sq = chnp.tile([P, SGB, T], F16, tag="v1sq")
                nc.scalar.activation(v1sq[:], v1[:], AF.Square,
                                     bias=prm(P_ZERO), scale=prm(P_SQ0))
                v2sq = chnp.tile([P, SGB, T], F16, tag="v2sq")
                nc.scalar.activation(v2sq[:], z[:, :, T:2 * T], AF.Square,
                                     bias=prm(P_ZERO), scale=prm(P_C3))

                n3 = chnp.tile([P, SGB, T], F16, tag="n3")
                nc.vector.tensor_add(n3[:], n2[:], it2[:])
                nc.gpsimd.tensor_add(n3[:], n3[:], v1sq[:])
                nc.vector.tensor_add(n3[:], n3[:], v2sq[:])
                inv3 = chnp.tile([P, SGB, T], F16, tag="inv3")
                nc.scalar.activation(inv3[:], n3[:], AF.Abs_reciprocal_sqrt,
                                     bias=prm(P_EPS), scale=1.0 / D)

                pa = chnp.tile([P, SGB, T], F16, tag="pa")
                s = chnp.tile([P, SGB, 2 * T], F16, tag="s")
                nc.vector.affine_then_add(pa[:], z[:, :, T:2 * T],
                                          ey_t[:, :, 2 * T:3 * T],
                                          scale=prm(P_H10), bias=0.0)
                nc.vector.affine_then_add(s[:, :, 0:T], z[:, :, 0:T], pa[:],
                                          scale=prm(P_H00), bias=0.0)
                pb = chnp.tile([P, SGB, T], F16, tag="pb")
                nc.vector.affine_then_add(pb[:], z[:, :, T:2 * T],
                                          ey_t[:, :, 3 * T:4 * T],
                                          scale=prm(P_H11), bias=0.0)
                nc.vector.affine_then_add(s[:, :, T:2 * T], z[:, :, 0:T], pb[:],
                                          scale=prm(P_H01), bias=0.0)
                # lint = s * inv3 (in place, per half)
                nc.gpsimd.tensor_mul(s[:, :, 0:T], s[:, :, 0:T], inv3[:])
                nc.vector.tensor_mul(s[:, :, T:2 * T], s[:, :, T:2 * T], inv3[:])

                # ---------------- phase F: transpose2 + matmul2 + store
                for g in range(SGB // 4):
                    pt2 = ptp.tile([2 * T, 4 * P], F16, tag="tp")
                    for b in range(4):
                        nc.tensor.transpose(pt2[:, b * P:(b + 1) * P],
                                            s[:, g * 4 + b, :], ident[:])
                    lt2 = dtp.tile([2 * T, 4 * P], F16, tag="lt2")
                    if g % 2 == 0:
                        nc.scalar.copy(lt2[:], pt2[:])
                    else:
                        nc.vector.tensor_copy(lt2[:], pt2[:])
                    for b in range(4):
                        jb = g * 4 + b
                        po = poutp.tile([P, N2], F32, tag="po")
                        nc.tensor.matmul(po[:], lt2[:, b * P:(b + 1) * P],
                                         rhs2_sb[:], start=True, stop=True)
                        o_sb = outp.tile([P, N2], F16, tag="osb")
                        if b % 2 == 0:
                            nc.scalar.copy(o_sb[:], po[:])
                        else:
                            nc.vector.tensor_copy(o_sb[:], po[:])
                        nc.sync.dma_start(out_v[:, j0 + jb, :], o_sb[:])

    nc.compile()
    return nc


_CACHE = {}


def _get_nc():
    if "nc" not in _CACHE:
        _CACHE["nc"] = build_bass()
    return _CACHE["nc"]


def make_uw(idx, LUTU, LUTW):
    """idx (B, T) int -> per-core fp16 basis [NCORES, P, NBLK, K1]."""
    u = LUTU[idx].astype(np.float16)   # (B, T)
    w = LUTW[idx].astype(np.float16)
    UW = np.empty((NCORES, P, NBLK, K1), dtype=np.float16)
    UW[..., 0:T] = u.reshape(NCORES, P, NBLK, T)
    UW[..., T:2 * T] = w.reshape(NCORES, P, NBLK, T)
    UW[..., 2 * T] = np.float16(1.0)
    return UW


def kernel(**inputs) -> np.ndarray:
    idx = np.asarray(inputs["idx"]).astype(np.int64)
    kw = {k: np.asarray(v, np.float64) for k, v in inputs.items() if k != "idx"}
    RQEY, RAR, RHS2, SROW, PRM, LUTU, LUTW = host_tables(**kw)
    UW = make_uw(idx, LUTU, LUTW)
    nc = _get_nc()
    in_maps = [
        {"uw": UW[c], "rqey": RQEY, "rar": RAR, "rhs2": RHS2,
         "srow": SROW, "prm": PRM}
        for c in range(NCORES)
    ]
    res = run_bass_kernel_spmd(nc, in_maps, core_ids=list(range(NCORES)))
    out = np.concatenate([res.results[c]["out"] for c in range(NCORES)], axis=0)
    return np.ascontiguousarray(out.reshape(B, T, V).astype(np.float32))


# revision 12
# speedup vs baseline: 2.8349x; 1.3013x over previous
"""Trainium2 Bass kernel for nn_MicroAdder (tiny dense transformer).

Decomposition (v2, fp16 PE path):
 - Host precomputes u_v=cos(ang_v), w_v=sin(ang_v) per vocab entry and expands
   idx -> basis [u_s, w_s, 1] (69 wide) directly into the fp16 "uw" input, so
   no trig runs on-device.
 - Per 128-row block: PE transpose basis -> [69,128], then two fp16 matmuls
   against constant matrices emit 8 per-element linear forms:
   QEY = (q0,q1,e0,e1,y0,y1) [204 cols] and AR = (alpha, r) [68 cols].
 - A short elementwise chain (2 abs_rsqrt, 2 square, ~15 mul/add) computes the
   two rmsnorm denominators and the head-plane values s0,s1 per (row,t).
 - PE transpose s -> [68,128], fp16 matmul vs block-diagonal RHS2 expands to
   logits [128,476]; fp16 copy to SBUF; fp16 DMA out; host casts to fp32.

Sharding: pure data parallel over the batch dim across 8 NeuronCores.
"""

import sys

import numpy as np

for _p in ("/opt/trn_rl_repo", "/root/.axon_site/_ro/trn_rl_repo"):
    if _p not in sys.path:
        sys.path.append(_p)

import concourse.bacc as bacc  # noqa: E402
import concourse.tile as tile  # noqa: E402
from concourse import mybir  # noqa: E402
from concourse.bass_utils import run_bass_kernel_spmd  # noqa: E402
from concourse.masks import make_identity  # noqa: E402

# ---------------------------------------------------------------- problem dims
B, T, V = 65536, 34, 14
D, EPS, MAX_DIGITS = 5, 1e-5, 10
NCORES = 8
BC = B // NCORES            # rows per core = 8192
P = 128                     # partitions
NBLK = BC // P              # blocks per core = 64 (block j = rows {p*NBLK + j})
SGB = 16                    # blocks per supergroup
NSG = NBLK // SGB           # 2 supergroups
K1 = 2 * T + 1              # basis size = 69
NQEY = 7 * T                # 238: q0,q1,e0,e1,y0,y1,alpha
NAR = T                     # 34: r
N2 = T * V                  # 476
NPRM = 10

F32 = mybir.dt.float32
F16 = mybir.dt.float16
AF = mybir.ActivationFunctionType
ALU = mybir.AluOpType

# PRM slots
P_RAT, P_SQ0, P_C3, P_H00, P_H10, P_H01, P_H11, P_EPS, P_ZERO, P_UNUSED = range(10)


# ---------------------------------------------------------------- host tables
def host_tables(tok_A, tok_start, tok_stride, sp_amp, sp_phase, sp_slope, sp_offset,
                norm_w, q_w, q_phase, out_A, out_B, fc1_w, fc2_w, head_w):
    f = np.float64
    A = f(tok_A)
    t = np.arange(T, dtype=f)
    th = 2.0 * np.pi * t / MAX_DIGITS + f(sp_phase)
    pos = np.stack([f(sp_amp) * np.cos(th), f(sp_amp) * np.sin(th),
                    f(sp_slope) * t + f(sp_offset)], axis=-1)
    k = pos @ np.asarray(q_w, f).T
    c0, s0 = np.cos(f(q_phase[0])), np.sin(f(q_phase[0]))
    q = k.copy()
    q[:, 0] = c0 * k[:, 0] - s0 * k[:, 1]
    q[:, 1] = s0 * k[:, 0] + c0 * k[:, 1]
    scores = (q @ k.T) / np.sqrt(f(5.0))
    sm = np.where(np.tril(np.ones((T, T), bool)), scores, -np.inf)
    sm = sm - sm.max(-1, keepdims=True)
    e = np.exp(sm)
    attn = e / e.sum(-1, keepdims=True)

    nw = np.asarray(norm_w, f)
    oA = np.asarray(out_A, f)[:, 0]
    oB = np.asarray(out_B, f)[0]
    S_t = A * A + (pos ** 2).sum(-1)
    rms1 = np.sqrt(S_t / D + EPS)

    M0 = attn * (A * nw[0] * oA[0] / rms1)[None, :]
    M1 = attn * (A * nw[1] * oA[1] / rms1)[None, :]
    c_t = attn @ ((pos * (nw[2:] * oA[2:])[None, :]).sum(-1) / rms1)

    g0 = np.asarray(fc2_w, f)[:, 0]
    g1 = np.asarray(fc2_w, f)[:, 1]

    dd = np.eye(T, dtype=f)
    b2 = (oB ** 2).sum()

    def lin_group(v):
        """Columns (69, T) for per-element value <v, x_t> with v a model-dim
        vector; x includes the rank-1 attention update."""
        g = np.zeros((K1, T), dtype=f)
        bv = (oB * v).sum()
        g[0:T] = A * v[0] * dd + bv * M0.T
        g[T:2 * T] = A * v[1] * dd + bv * M1.T
        g[2 * T] = (pos * v[None, 2:]).sum(-1) + bv * c_t
        return g

    def alpha_group():
        g = np.zeros((K1, T), dtype=f)
        g[0:T] = M0.T
        g[T:2 * T] = M1.T
        g[2 * T] = c_t
        return g

    def r_group():
        # r = 2<x_emb, oB> + b2 * alpha  (so that alpha*r = |x|^2 - |x_emb|^2)
        g = np.zeros((K1, T), dtype=f)
        g[0:T] = 2 * A * oB[0] * dd + b2 * M0.T
        g[T:2 * T] = 2 * A * oB[1] * dd + b2 * M1.T
        g[2 * T] = 2 * (pos * oB[None, 2:]).sum(-1) + b2 * c_t
        return g

    RQEY = np.concatenate([
        lin_group(nw * np.asarray(fc1_w, f)[0]),      # q0
        lin_group(nw * np.asarray(fc1_w, f)[1]),      # q1
        lin_group(2.0 * g0),                          # e0
        lin_group(2.0 * g1),                          # e1
        lin_group(nw * np.asarray(head_w, f)[0]),     # y0
        lin_group(nw * np.asarray(head_w, f)[1]),     # y1
        alpha_group(),                                # alpha
    ], axis=1)
    RAR = r_group()

    G00, G01, G11 = (g0 * g0).sum(), (g0 * g1).sum(), (g1 * g1).sum()
    if G00 > 1e-30:
        sq0, rat = np.sqrt(G00), G01 / G00
        c3 = np.sqrt(max(G11 - G01 * G01 / G00, 0.0))
    else:
        sq0, rat, c3 = 0.0, 0.0, np.sqrt(G11)
    hv0 = nw * np.asarray(head_w, f)[0]
    hv1 = nw * np.asarray(head_w, f)[1]
    H = np.array([[(g0 * hv0).sum(), (g0 * hv1).sum()],
                  [(g1 * hv0).sum(), (g1 * hv1).sum()]])

    dvoc = np.arange(V, dtype=f)
    ang = f(tok_start) + dvoc * f(tok_stride)
    E = np.stack([A * np.cos(ang), A * np.sin(ang)], axis=-1)
    RHS2 = np.zeros((2 * T, N2), dtype=f)
    for t_ in range(T):
        RHS2[t_, t_ * V:(t_ + 1) * V] = E[:, 0]
        RHS2[T + t_, t_ * V:(t_ + 1) * V] = E[:, 1]

    SROW = np.tile(S_t, SGB)[None, :]  # [1, SGB*T]
    PRM = np.zeros((1, NPRM), dtype=np.float64)
    PRM[0, P_RAT] = rat
    PRM[0, P_SQ0] = sq0
    PRM[0, P_C3] = c3
    PRM[0, P_H00] = H[0, 0]
    PRM[0, P_H10] = H[1, 0]
    PRM[0, P_H01] = H[0, 1]
    PRM[0, P_H11] = H[1, 1]
    PRM[0, P_EPS] = EPS
    PRM[0, P_ZERO] = 0.0

    LUTU = np.cos(ang).astype(np.float32)
    LUTW = np.sin(ang).astype(np.float32)
    return (RQEY.astype(np.float16), RAR.astype(np.float16),
            RHS2.astype(np.float16), SROW.astype(np.float16).copy(),
            PRM.astype(np.float32).copy(), LUTU, LUTW)


# ---------------------------------------------------------------- bass kernel
def build_bass():
    nc = bacc.Bacc("TRN2", target_bir_lowering=False, debug=False)

    uw_d = nc.dram_tensor("uw", [P, NBLK, K1], F16, kind="ExternalInput").ap()
    rqey_d = nc.dram_tensor("rqey", [K1, NQEY], F16, kind="ExternalInput").ap()
    rar_d = nc.dram_tensor("rar", [K1, NAR], F16, kind="ExternalInput").ap()
    rhs2_d = nc.dram_tensor("rhs2", [2 * T, N2], F16, kind="ExternalInput").ap()
    srow_d = nc.dram_tensor("srow", [1, SGB * T], F16, kind="ExternalInput").ap()
    prm_d = nc.dram_tensor("prm", [1, NPRM], F32, kind="ExternalInput").ap()
    out_d = nc.dram_tensor("out", [BC, N2], F16, kind="ExternalOutput").ap()

    out_v = out_d.rearrange("(p n) c -> p n c", p=P)       # [128, 64, 476]

    E1 = SGB * T  # 1088 chain width per supergroup

    with tile.TileContext(nc) as tc:
        with (
            tc.tile_pool(name="const", bufs=1) as cpool,
            tc.tile_pool(name="uwp", bufs=4) as uwp,
            tc.tile_pool(name="dtp", bufs=2) as dtp,
            tc.tile_pool(name="stg", bufs=3) as stgp,
            tc.tile_pool(name="chn", bufs=3) as chnp,
            tc.tile_pool(name="outsb", bufs=4) as outp,
            tc.tile_pool(name="ptp", bufs=1, space="PSUM") as ptp,
            tc.tile_pool(name="qeyp", bufs=2, space="PSUM") as qeyp,
            tc.tile_pool(name="arp", bufs=1, space="PSUM") as arp,
            tc.tile_pool(name="pout", bufs=2, space="PSUM") as poutp,
        ):
            # ---- constants
            ident = cpool.tile([P, P], F16)
            make_identity(nc, ident[:])
            rqey_sb = cpool.tile([K1, NQEY], F16)
            nc.sync.dma_start(rqey_sb[:], rqey_d)
            rar_sb = cpool.tile([K1, NAR], F16)
            nc.sync.dma_start(rar_sb[:], rar_d)
            rhs2_sb = cpool.tile([2 * T, N2], F16)
            nc.sync.dma_start(rhs2_sb[:], rhs2_d)
            srow_sb = cpool.tile([P, SGB * T], F16)
            nc.sync.dma_start(srow_sb[:], srow_d.broadcast_to([P, SGB * T]))
            prm_sb = cpool.tile([P, NPRM], F32)
            nc.sync.dma_start(prm_sb[:], prm_d.broadcast_to([P, NPRM]))

            def prm(i):
                return prm_sb[:, i:i + 1]

            uw_ts, stg_ts, chn_ts = {}, {}, {}

            def phase_dma(sg):
                j0 = sg * SGB
                uw_t = uwp.tile([P, SGB, K1], F16, tag="uw", name=f"uw{sg}")
                nc.sync.dma_start(uw_t[:], uw_d[:, j0:j0 + SGB, :])
                uw_ts[sg] = uw_t

            def phase_b(sg):
                uw_t = uw_ts[sg]
                rho_t = stgp.tile([P, SGB, 2 * T], F16, tag="rho", name=f"rho{sg}")
                ey_t = stgp.tile([P, SGB, 5 * T], F16, tag="eyt", name=f"ey{sg}")
                ar_t = stgp.tile([P, SGB, T], F16, tag="art", name=f"ar{sg}")
                stg_ts[sg] = (rho_t, ey_t, ar_t)
                for g in range(SGB // 4):
                    pt1 = ptp.tile([K1, 4 * P], F16, tag="tp", name=f"pt1_{sg}_{g}")
                    for b in range(4):
                        nc.tensor.transpose(pt1[:, b * P:(b + 1) * P],
                                            uw_t[:, g * 4 + b, :], ident[:])
                    dt = dtp.tile([K1, 4 * P], F16, tag="dt", name=f"dt{sg}_{g}")
                    nc.vector.tensor_copy(dt[:], pt1[:])
                    qey = qeyp.tile([P, 4, 256], F32, tag="qey", name=f"qey{sg}_{g}")
                    arq = arp.tile([P, 4 * NAR], F32, tag="ar", name=f"arq{sg}_{g}")
                    qv = qey[:, :, 0:NQEY]
                    av = arq[:].rearrange("p (b c) -> p b c", b=4)
                    for b in range(4):
                        nc.tensor.matmul(qey[:, b, 0:NQEY],
                                         dt[:, b * P:(b + 1) * P], rqey_sb[:],
                                         start=True, stop=True)
                        nc.tensor.matmul(arq[:, b * NAR:(b + 1) * NAR],
                                         dt[:, b * P:(b + 1) * P], rar_sb[:],
                                         start=True, stop=True)
                    sl = slice(g * 4, g * 4 + 4)
                    nc.scalar.activation(rho_t[:, sl, :], qv[:, :, 0:2 * T], AF.Relu,
                                         bias=prm(P_ZERO), scale=1.0)
                    nc.scalar.copy(ey_t[:, sl, :], qv[:, :, 2 * T:7 * T])
                    nc.vector.tensor_mul(ar_t[:, sl, :], ey_t[:, sl, 4 * T:5 * T],
                                         av[:, :, 0:T])

            def phase_d(sg):
                rho_t, ey_t, ar_t = stg_ts[sg]
                nm = lambda base: f"{base}{sg}"  # noqa: E731
                n2 = chnp.tile([P, SGB, T], F16, tag="n2", name=nm("n2"))
                nc.vector.tensor_add(n2[:], ar_t[:],
                                     srow_sb[:].rearrange("p (a t) -> p a t", t=T))
                inv2 = chnp.tile([P, SGB, T], F16, tag="inv2", name=nm("inv2"))
                nc.scalar.activation(inv2[:], n2[:], AF.Abs_reciprocal_sqrt,
                                     bias=prm(P_EPS), scale=1.0 / D)
                z = chnp.tile([P, SGB, 2 * T], F16, tag="z", name=nm("z"))
                nc.vector.tensor_mul(z[:, :, 0:T], rho_t[:, :, 0:T], inv2[:])
                nc.vector.tensor_mul(z[:, :, T:2 * T], rho_t[:, :, T:2 * T], inv2[:])
                ze = chnp.tile([P, SGB, 2 * T], F16, tag="ze", name=nm("ze"))
                nc.vector.tensor_mul(ze[:], z[:], ey_t[:, :, 0:2 * T])
                it2 = chnp.tile([P, SGB, T], F16, tag="it2", name=nm("it2"))
                nc.gpsimd.tensor_add(it2[:], ze[:, :, 0:T], ze[:, :, T:2 * T])

                v1 = chnp.tile([P, SGB, T], F16, tag="v1", name=nm("v1"))
                nc.vector.scalar_tensor_tensor(v1[:], z[:, :, T:2 * T], prm(P_RAT),
                                               z[:, :, 0:T], op0=ALU.mult,
                                               op1=ALU.add)
                v1sq = chnp.tile([P, SGB, T], F16, tag="v1sq", name=nm("v1sq"))
                nc.scalar.activation(v1sq[:], v1[:], AF.Square,
                                     bias=prm(P_ZERO), scale=prm(P_SQ0))
                v2sq = chnp.tile([P, SGB, T], F16, tag="v2sq", name=nm("v2sq"))
                nc.scalar.activation(v2sq[:], z[:, :, T:2 * T], AF.Square,
                                     bias=prm(P_ZERO), scale=prm(P_C3))

                n3 = chnp.tile([P, SGB, T], F16, tag="n3", name=nm("n3"))
                nc.vector.tensor_add(n3[:], n2[:], v1sq[:])
                nc.vector.tensor_add(n3[:], n3[:], v2sq[:])
                nc.vector.tensor_add(n3[:], n3[:], it2[:])
                inv3 = chnp.tile([P, SGB, T], F16, tag="inv3", name=nm("inv3"))
                nc.scalar.activation(inv3[:], n3[:], AF.Abs_reciprocal_sqrt,
                                     bias=prm(P_EPS), scale=1.0 / D)

                pa = chnp.tile([P, SGB, T], F16, tag="pa", name=nm("pa"))
                s = chnp.tile([P, SGB, 2 * T], F16, tag="s", name=nm("s"))
                nc.vector.scalar_tensor_tensor(pa[:], z[:, :, T:2 * T], prm(P_H10),
                                               ey_t[:, :, 2 * T:3 * T],
                                               op0=ALU.mult, op1=ALU.add)
                nc.vector.scalar_tensor_tensor(s[:, :, 0:T], z[:, :, 0:T],
                                               prm(P_H00), pa[:],
                                               op0=ALU.mult, op1=ALU.add)
                pb = chnp.tile([P, SGB, T], F16, tag="pb", name=nm("pb"))
                nc.vector.scalar_tensor_tensor(pb[:], z[:, :, T:2 * T], prm(P_H11),
                                               ey_t[:, :, 3 * T:4 * T],
                                               op0=ALU.mult, op1=ALU.add)
                nc.vector.scalar_tensor_tensor(s[:, :, T:2 * T], z[:, :, 0:T],
                                               prm(P_H01), pb[:],
                                               op0=ALU.mult, op1=ALU.add)
                nc.vector.tensor_mul(s[:, :, 0:T], s[:, :, 0:T], inv3[:])
                nc.vector.tensor_mul(s[:, :, T:2 * T], s[:, :, T:2 * T], inv3[:])
                chn_ts[sg] = s

            def phase_f(sg):
                j0 = sg * SGB
                s = chn_ts[sg]
                for g in range(SGB // 4):
                    pt2 = ptp.tile([2 * T, 4 * P], F16, tag="tp", name=f"pt2_{sg}_{g}")
                    for b in range(4):
                        nc.tensor.transpose(pt2[:, b * P:(b + 1) * P],
                                            s[:, g * 4 + b, :], ident[:])
                    lt2 = dtp.tile([2 * T, 4 * P], F16, tag="lt2", name=f"lt2{sg}_{g}")
                    nc.vector.tensor_copy(lt2[:], pt2[:])
                    for b in range(4):
                        jb = g * 4 + b
                        po = poutp.tile([P, N2], F32, tag="po", name=f"po{sg}_{g}_{b}")
                        nc.tensor.matmul(po[:], lt2[:, b * P:(b + 1) * P],
                                         rhs2_sb[:], start=True, stop=True)
                        o_sb = outp.tile([P, N2], F16, tag="osb", name=f"o{sg}_{jb}")
                        if b % 2 == 0:
                            nc.scalar.copy(o_sb[:], po[:])
                        else:
                            nc.vector.tensor_copy(o_sb[:], po[:])
                        nc.sync.dma_start(out_v[:, j0 + jb, :], o_sb[:])

            # software pipeline: keep PE queue free of waits on the D chain
            phase_dma(0)
            phase_dma(1)
            phase_b(0)
            phase_dma(2)
            phase_b(1)
            phase_d(0)
            phase_dma(3)
            phase_b(2)
            phase_d(1)
            phase_f(0)
            phase_b(3)
            phase_d(2)
            phase_f(1)
            phase_d(3)
            phase_f(2)
            phase_f(3)

    nc.compile()
    return nc


_CACHE = {}


def _get_nc():
    if "nc" not in _CACHE:
        _CACHE["nc"] = build_bass()
    return _CACHE["nc"]


def make_uw(idx, LUTU, LUTW):
    """idx (B, T) int -> per-core fp16 basis [NCORES, P, NBLK, K1]."""
    u = LUTU[idx].astype(np.float16)   # (B, T)
    w = LUTW[idx].astype(np.float16)
    UW = np.empty((NCORES, P, NBLK, K1), dtype=np.float16)
    UW[..., 0:T] = u.reshape(NCORES, P, NBLK, T)
    UW[..., T:2 * T] = w.reshape(NCORES, P, NBLK, T)
    UW[..., 2 * T] = np.float16(1.0)
    return UW


def kernel(**inputs) -> np.ndarray:
    idx = np.asarray(inputs["idx"]).astype(np.int64)
    kw = {k: np.asarray(v, np.float64) for k, v in inputs.items() if k != "idx"}
    RQEY, RAR, RHS2, SROW, PRM, LUTU, LUTW = host_tables(**kw)
    UW = make_uw(idx, LUTU, LUTW)
    nc = _get_nc()
    in_maps = [
        {"uw": UW[c], "rqey": RQEY, "rar": RAR, "rhs2": RHS2,
         "srow": SROW, "prm": PRM}
        for c in range(NCORES)
    ]
    res = run_bass_kernel_spmd(nc, in_maps, core_ids=list(range(NCORES)))
    out = np.concatenate([res.results[c]["out"] for c in range(NCORES)], axis=0)
    return np.ascontiguousarray(out.reshape(B, T, V).astype(np.float32))
